# revision 6
# baseline (speedup 1.0000x reference)
"""Trainium2 Bass kernel for nn_BasisDense: y = einsum('bd,duk,bk->bu', x, kernel, c_prob) + bias.

Strategy (orthogonal K-rotation, 7-expert fp8 delta path):
  Rotate the K dim by orthogonal Q whose first column is the top right-singular
  vector v of c_prob:  c' = c_prob @ Q,  K'_j = sum_k kernel[:,:,k] Q[k,j].
  Then  y = sum_j c'_j (x @ K'_j) + bias  exactly, and:
    j=0 ("sum path"): carries ~78% of output variance -> bf16, U columns.
       Host folds a=c'_0 into x (xsum = a*x), bias folded in at PSUM drain.
    j=1..7 ("delta path"): c'_j are small residuals -> e4m3 DoubleRow
       (0.5 cyc/row) over fused (u, 7) columns; quantization error attenuated
       by sqrt(resid energy ratio ~0.22) stays within tolerance.
  Streamed PE columns: (2 + 7)/16 units vs 14/16 for the old mixed scheme.
  - Hybrid shard across 8 cores: batch B into 4 x units U into 2.
  - Sum path (256 bf16 MMs) runs first, t-outer/bt-inner so fill-phase demand
    matches DMA supply; doubles as PE warm-up.  Delta path: 896 DR MMs of 448
    cols, one PSUM accumulation group per (chunk-half, bt); epilogue multiplies
    by a 64x-replicated c'/(sx*sk) and reduces over the 7 rotated experts.
  - Host-side input marshaling: transposes/casts/packing + one [D*U,8]@[8,8]
    rotation GEMM.
"""
import sys

sys.path.insert(0, "/opt/trn_rl_repo")

import numpy as np
import concourse.bacc as bacc
import concourse.mybir as mybir
import concourse.tile as tile
from concourse import bass_utils

B, D, U, K = 4096, 2048, 2048, 8
NCORES = 8
SHARD_U = 2  # units-dimension shards
SHARD_B = NCORES // SHARD_U
BS = B // SHARD_B  # batch rows per core
USH = U // SHARD_U  # units per core
KE = K - 1  # delta-path experts after rotation
UKS = USH * KE  # fused (u, j) output columns per core
NFREE = 512  # sum-path matmul free dim / PSUM bank stride
DFREE = 64 * KE  # delta-path matmul free dim (448)
NW = 2 * DFREE  # kern8 chunk width (896)
NG = UKS // NW  # kern8 chunk groups (8)
DT = D // 128  # contraction tiles
BT = BS // 128  # batch partition-tiles per core
UPT = DFREE // KE  # u-columns produced per fused n-tile (64)
UOT = USH // NFREE  # sum-path u tiles per bt
KT_BUFS = 3
SX = 4.0  # x -> e4m3 pre-scale (sigma ~4)
SK = 200.0  # kernel -> e4m3 pre-scale (sigma ~4)

_CACHE = {}


def _build():
    nc = bacc.Bacc("TRN2", target_bir_lowering=False, debug=False, num_devices=NCORES)
    f32 = mybir.dt.float32
    bf16 = mybir.dt.bfloat16
    f8 = mybir.dt.float8e4

    xsum = nc.dram_tensor("xsum", [D, BS], bf16, kind="ExternalInput").ap()
    x8 = nc.dram_tensor("x8", [128, DT, BS], f8, kind="ExternalInput").ap()
    dcp = nc.dram_tensor("dcp", [128, BT * KE], f32, kind="ExternalInput").ap()
    ksum = nc.dram_tensor("ksum", [128, DT, USH], bf16, kind="ExternalInput").ap()
    kern8 = nc.dram_tensor("kern8", [128, DT, UKS], f8, kind="ExternalInput").ap()
    biasr = nc.dram_tensor("biasr", [128, USH], f32, kind="ExternalInput").ap()
    y = nc.dram_tensor("y", [BS, USH], f32, kind="ExternalOutput").ap()

    with tile.TileContext(nc) as tc:
        with (
            tc.tile_pool(name="const", bufs=1) as constp,
            tc.tile_pool(name="kt", bufs=KT_BUFS) as ktp,
            tc.tile_pool(name="mps", bufs=8, space="PSUM") as mps,
            tc.tile_pool(name="ep", bufs=4) as epp,
            tc.tile_pool(name="yp", bufs=8) as ypp,
        ):
            xsT = constp.tile([128, DT, BS], bf16)  # [d-part, d-tile, b] a-scaled
            x8T = constp.tile([128, DT, BS], f8)
            ksT = constp.tile([128, DT, USH], bf16)
            ysum = constp.tile([128, BT, USH], bf16)  # sum-path out + bias
            dc_rep = constp.tile([128, BT, DFREE], f32)
            bias_rep = constp.tile([128, USH], f32)

            xsum_v = xsum.rearrange("(t p) b -> p t b", p=128)
            dc_nat = constp.tile([128, BT, KE], f32)
            nc.gpsimd.dma_start(bias_rep, biasr)
            # per-t DMAs: sum path MMs chase these as they land.  xsT/ksT ride
            # the two HWDGE queues (needed first); x8T rides gpsimd (SWDGE,
            # needed only when the delta path starts)
            # fill-critical split: the uo=0 sweep reads only ksT[:, :, 0:512],
            # so ship those halves + xsT first; ksT second halves stream later
            # under the uo=0 compute window.  The first tiles are additionally
            # partition-split across both queues to halve their latency.
            NSPLIT = 4
            for t in range(DT):
                eng = nc.sync if t % 2 == 0 else nc.scalar
                eng2 = nc.scalar if t % 2 == 0 else nc.sync
                if t < NSPLIT:
                    eng.dma_start(ksT[0:64, t, 0:NFREE], ksum[0:64, t, 0:NFREE])
                    eng2.dma_start(ksT[64:128, t, 0:NFREE], ksum[64:128, t, 0:NFREE])
                    eng.dma_start(xsT[0:64, t, :], xsum_v[0:64, t, :])
                    eng2.dma_start(xsT[64:128, t, :], xsum_v[64:128, t, :])
                else:
                    eng.dma_start(ksT[:, t, 0:NFREE], ksum[:, t, 0:NFREE])
                    eng2.dma_start(xsT[:, t, :], xsum_v[:, t, :])
            nc.scalar.dma_start(dc_nat, dcp.rearrange("p (bt k) -> p bt k", k=KE))
            for t in range(DT):
                eng = nc.sync if t % 2 == 0 else nc.scalar
                eng.dma_start(ksT[:, t, NFREE:USH], ksum[:, t, NFREE:USH])
            for t in range(DT):
                nc.gpsimd.dma_start(x8T[:, t, :], x8[:, t, :])
            # replicate c'/(sx*sk) 64x along free dim on the DVE (tiny)
            for bt in range(BT):
                nc.vector.tensor_copy(dc_rep[:, bt, 0:KE], dc_nat[:, bt, :])
                s = KE
                while s < DFREE:
                    e = min(2 * s, DFREE)
                    nc.vector.tensor_copy(dc_rep[:, bt, s:e], dc_rep[:, bt, 0 : e - s])
                    s = e

            # dummy matmuls on a zeroed tile: warms the PE HAM clock-gate
            # during the DMA fill phase, before real data lands
            wz = constp.tile([128, NFREE + 128], bf16)
            nc.vector.memset(wz, 0)
            warm = mps.tile([128, NFREE], f32, tag="acc", name="warm")
            for _ in range(4):
                nc.tensor.matmul(
                    warm, wz[:, NFREE:], wz[:, 0:NFREE], start=True, stop=True
                )

            # ---- sum path: ysum[bt, u] = (a*x) @ ksum + bias, drained as bf16
            # t-outer/bt-inner: each (xsT, ksT) d-tile pair feeds 8 MMs, so
            # the fill-phase demand rate matches the DMA supply rate
            for uo in range(UOT):
                saccs = [
                    mps.tile([128, NFREE], f32, tag="acc", name=f"sacc{uo}_{bt}")
                    for bt in range(BT)
                ]
                for t in range(DT):
                    for bt in range(BT):
                        nc.tensor.matmul(
                            saccs[bt],
                            xsT[:, t, bt * 128 : (bt + 1) * 128],
                            ksT[:, t, uo * NFREE : (uo + 1) * NFREE],
                            start=(t == 0),
                            stop=(t == DT - 1),
                        )
                for bt in range(BT):
                    nc.vector.tensor_add(
                        ysum[:, bt, uo * NFREE : (uo + 1) * NFREE],
                        saccs[bt],
                        bias_rep[:, uo * NFREE : (uo + 1) * NFREE],
                    )

            # ---- delta path: 7 rotated experts, ALL-fp8 DoubleRow
            def epilogue(acc, bt, n):
                # y[b,u] = sum_j acc[b,(u,j)]*dc[b,j] + ysum[b,u]
                tmp = epp.tile([128, DFREE], f32, tag="tmp")
                nc.vector.tensor_mul(tmp, acc, dc_rep[:, bt, :])
                yt = ypp.tile([128, UPT], f32, tag="yt")
                nc.vector.tensor_reduce(
                    yt,
                    tmp.rearrange("p (u k) -> p u k", k=KE),
                    axis=mybir.AxisListType.X,
                    op=mybir.AluOpType.add,
                )
                yf = ypp.tile([128, UPT], f32, tag="yf")
                nc.vector.tensor_add(yf, yt, ysum[:, bt, n * UPT : (n + 1) * UPT])
                nc.scalar.dma_start(
                    y[bt * 128 : (bt + 1) * 128, n * UPT : (n + 1) * UPT],
                    yf,
                )

            for g in range(NG):
                kt = ktp.tile([128, DT, NW], f8, tag="kt")
                for t in range(DT):
                    eng = nc.sync if t % 2 == 0 else nc.scalar
                    eng.dma_start(kt[:, t, :], kern8[:, t, g * NW : (g + 1) * NW])
                for half in range(NW // DFREE):
                    hs = half * DFREE
                    for bt in range(BT):
                        # full-bank tile, matmul writes the 448-col subview
                        accb = mps.tile([128, NFREE], f32, tag="acc", name="acc")
                        acc = accb[:, 0:DFREE]
                        for j in range(DT // 2):
                            nc.tensor.matmul(
                                acc,
                                x8T[:, 2 * j : 2 * j + 2, bt * 128 : (bt + 1) * 128],
                                kt[:, 2 * j : 2 * j + 2, hs : hs + DFREE],
                                start=(j == 0),
                                stop=(j == DT // 2 - 1),
                                perf_mode=mybir.MatmulPerfMode.DoubleRow,
                                skip_group_check=True,
                            )
                        epilogue(acc, bt, 2 * g + half)
    nc.compile()
    return nc


def _in_maps(x, c_prob, kernel, bias):
    import ml_dtypes

    bf16 = ml_dtypes.bfloat16
    e4m3 = ml_dtypes.float8_e4m3
    x = np.ascontiguousarray(x, dtype=np.float32)
    c_prob = np.ascontiguousarray(c_prob, dtype=np.float32)
    kernel = np.ascontiguousarray(kernel, dtype=np.float32)
    bias = np.ascontiguousarray(bias, dtype=np.float32)

    # orthogonal rotation with top right-singular vector of c_prob first
    _, _, vt = np.linalg.svd(c_prob, full_matrices=False)
    v = vt[0].astype(np.float32)
    if v.sum() < 0:
        v = -v
    M = np.concatenate([v[:, None], np.eye(K, dtype=np.float32)], axis=1)
    Qf, _ = np.linalg.qr(M)
    Q = Qf[:, :K].astype(np.float32)
    if np.dot(Q[:, 0], v) < 0:
        Q = -Q

    cr = c_prob @ Q  # [B, K]; cr[:,0] = a
    a = cr[:, 0]
    dc = cr[:, 1:] * np.float32(1.0 / (SX * SK))  # [B, KE]

    kr = (kernel.reshape(D * U, K) @ Q).reshape(D, U, K)
    kv_full = np.ascontiguousarray(kr[:, :, 0]).astype(bf16)  # [D, U]
    k7_full = (np.ascontiguousarray(kr[:, :, 1:]) * SK).astype(e4m3)  # [D, U, KE]
    xs_full = (x * a[:, None]).astype(bf16)
    x8_full = (x * SX).astype(e4m3)

    maps = []
    for c in range(NCORES):
        bq, uh = c % SHARD_B, c // SHARD_B
        bsl = slice(bq * BS, (bq + 1) * BS)
        usl = slice(uh * USH, (uh + 1) * USH)
        dcpk = np.ascontiguousarray(
            dc[bsl].reshape(BT, 128, KE).transpose(1, 0, 2).reshape(128, BT * KE)
        )
        x8p = np.ascontiguousarray(
            x8_full[bsl].T.reshape(DT, 128, BS).transpose(1, 0, 2)
        )
        ksp = np.ascontiguousarray(
            kv_full[:, usl].reshape(DT, 128, USH).transpose(1, 0, 2)
        )
        k8p = np.ascontiguousarray(
            k7_full[:, usl, :].reshape(DT, 128, UKS).transpose(1, 0, 2)
        )
        maps.append(
            {
                "xsum": np.ascontiguousarray(xs_full[bsl].T),
                "x8": x8p,
                "dcp": dcpk,
                "ksum": ksp,
                "kern8": k8p,
                "biasr": np.ascontiguousarray(
                    np.broadcast_to(bias[usl], (128, USH))
                ),
            }
        )
    return maps


def kernel(x, c_prob, kernel, bias):
    if "nc" not in _CACHE:
        _CACHE["nc"] = _build()
    nc = _CACHE["nc"]
    res = bass_utils.run_bass_kernel_spmd(
        nc, _in_maps(x, c_prob, kernel, bias), list(range(NCORES))
    )
    out = np.empty((B, U), dtype=np.float32)
    for c in range(NCORES):
        bq, uh = c % SHARD_B, c // SHARD_B
        out[bq * BS : (bq + 1) * BS, uh * USH : (uh + 1) * USH] = res.results[c]["y"]
    return out


# revision 7
# speedup vs baseline: 1.0075x; 1.0075x over previous
"""Trainium2 Bass kernel for nn_BasisDense: y = einsum('bd,duk,bk->bu', x, kernel, c_prob) + bias.

Strategy (orthogonal K-rotation, 7-expert fp8 delta path):
  Rotate the K dim by orthogonal Q whose first column is the top right-singular
  vector v of c_prob:  c' = c_prob @ Q,  K'_j = sum_k kernel[:,:,k] Q[k,j].
  Then  y = sum_j c'_j (x @ K'_j) + bias  exactly, and:
    j=0 ("sum path"): carries ~78% of output variance -> bf16, U columns.
       Host folds a=c'_0 into x (xsum = a*x), bias folded in at PSUM drain.
    j=1..7 ("delta path"): c'_j are small residuals -> e4m3 DoubleRow
       (0.5 cyc/row) over fused (u, 7) columns; quantization error attenuated
       by sqrt(resid energy ratio ~0.22) stays within tolerance.
  Streamed PE columns: (2 + 7)/16 units vs 14/16 for the old mixed scheme.
  - Hybrid shard across 8 cores: batch B into 4 x units U into 2.
  - Sum path (256 bf16 MMs) runs first, t-outer/bt-inner so fill-phase demand
    matches DMA supply; doubles as PE warm-up.  Delta path: 896 DR MMs of 448
    cols, one PSUM accumulation group per (chunk-half, bt); epilogue multiplies
    by a 64x-replicated c'/(sx*sk) and reduces over the 7 rotated experts.
  - Host-side input marshaling: transposes/casts/packing + one [D*U,8]@[8,8]
    rotation GEMM.
"""
import sys

sys.path.insert(0, "/opt/trn_rl_repo")

import numpy as np
import concourse.bacc as bacc
import concourse.mybir as mybir
import concourse.tile as tile
from concourse import bass_utils

B, D, U, K = 4096, 2048, 2048, 8
NCORES = 8
SHARD_U = 2  # units-dimension shards
SHARD_B = NCORES // SHARD_U
BS = B // SHARD_B  # batch rows per core
USH = U // SHARD_U  # units per core
KE = K - 1  # delta-path experts after rotation
UKS = USH * KE  # fused (u, j) output columns per core
NFREE = 512  # sum-path matmul free dim / PSUM bank stride
DFREE = 64 * KE  # delta-path matmul free dim (448)
NW = 2 * DFREE  # kern8 chunk width (896)
NG = UKS // NW  # kern8 chunk groups (8)
DT = D // 128  # contraction tiles
BT = BS // 128  # batch partition-tiles per core
UPT = DFREE // KE  # u-columns produced per fused n-tile (64)
UOT = USH // NFREE  # sum-path u tiles per bt
KT_BUFS = 3
SX = 4.0  # x -> e4m3 pre-scale (sigma ~4)
SK = 200.0  # kernel -> e4m3 pre-scale (sigma ~4)

_CACHE = {}


def _build():
    nc = bacc.Bacc("TRN2", target_bir_lowering=False, debug=False, num_devices=NCORES)
    f32 = mybir.dt.float32
    bf16 = mybir.dt.bfloat16
    f8 = mybir.dt.float8e4

    xsum = nc.dram_tensor("xsum", [D, BS], bf16, kind="ExternalInput").ap()
    x8 = nc.dram_tensor("x8", [128, DT, BS], f8, kind="ExternalInput").ap()
    dcp = nc.dram_tensor("dcp", [128, BT * KE], f32, kind="ExternalInput").ap()
    ksum = nc.dram_tensor("ksum", [128, DT, USH], bf16, kind="ExternalInput").ap()
    kern8 = nc.dram_tensor("kern8", [128, DT, UKS], f8, kind="ExternalInput").ap()
    biasr = nc.dram_tensor("biasr", [128, USH], f32, kind="ExternalInput").ap()
    y = nc.dram_tensor("y", [BS, USH], f32, kind="ExternalOutput").ap()

    with tile.TileContext(nc) as tc:
        with (
            tc.tile_pool(name="const", bufs=1) as constp,
            tc.tile_pool(name="kt", bufs=KT_BUFS) as ktp,
            tc.tile_pool(name="mps", bufs=8, space="PSUM") as mps,
            tc.tile_pool(name="ep", bufs=4) as epp,
            tc.tile_pool(name="yp", bufs=8) as ypp,
        ):
            xsT = constp.tile([128, DT, BS], bf16)  # [d-part, d-tile, b] a-scaled
            x8T = constp.tile([128, DT, BS], f8)
            ksT = constp.tile([128, DT, USH], bf16)
            ysum = constp.tile([128, BT, USH], bf16)  # sum-path out + bias
            dc_rep = constp.tile([128, BT, DFREE], f32)
            bias_rep = constp.tile([128, USH], f32)

            xsum_v = xsum.rearrange("(t p) b -> p t b", p=128)
            dc_nat = constp.tile([128, BT, KE], f32)
            nc.scalar.dma_start(dc_nat, dcp.rearrange("p (bt k) -> p bt k", k=KE))
            nc.gpsimd.dma_start(bias_rep, biasr)
            # per-t DMAs: sum path MMs chase these as they land.  xsT/ksT ride
            # the two HWDGE queues (needed first); x8T rides gpsimd (SWDGE,
            # needed only when the delta path starts)
            # fill-critical split: the uo=0 sweep reads only ksT[:, :, 0:512],
            # so ship those halves + xsT first; ksT second halves stream later
            # under the uo=0 compute window
            for t in range(DT):
                eng = nc.sync if t % 2 == 0 else nc.scalar
                eng.dma_start(ksT[:, t, 0:NFREE], ksum[:, t, 0:NFREE])
                eng2 = nc.scalar if t % 2 == 0 else nc.sync
                eng2.dma_start(xsT[:, t, :], xsum_v[:, t, :])
            for t in range(DT):
                eng = nc.sync if t % 2 == 0 else nc.scalar
                eng.dma_start(ksT[:, t, NFREE:USH], ksum[:, t, NFREE:USH])
            for t in range(DT):
                nc.gpsimd.dma_start(x8T[:, t, :], x8[:, t, :])
            # replicate c'/(sx*sk) 64x along free dim on the DVE (tiny)
            for bt in range(BT):
                nc.vector.tensor_copy(dc_rep[:, bt, 0:KE], dc_nat[:, bt, :])
                s = KE
                while s < DFREE:
                    e = min(2 * s, DFREE)
                    nc.vector.tensor_copy(dc_rep[:, bt, s:e], dc_rep[:, bt, 0 : e - s])
                    s = e

            # dummy matmuls on a zeroed tile: warms the PE HAM clock-gate
            # during the DMA fill phase, before real data lands
            wz = constp.tile([128, NFREE + 128], bf16)
            nc.vector.memset(wz, 0)
            warm = mps.tile([128, NFREE], f32, tag="acc", name="warm")
            for _ in range(4):
                nc.tensor.matmul(
                    warm, wz[:, NFREE:], wz[:, 0:NFREE], start=True, stop=True
                )

            # ---- sum path: ysum[bt, u] = (a*x) @ ksum + bias, drained as bf16
            # t-outer/bt-inner: each (xsT, ksT) d-tile pair feeds 8 MMs, so
            # the fill-phase demand rate matches the DMA supply rate
            for uo in range(UOT):
                saccs = [
                    mps.tile([128, NFREE], f32, tag="acc", name=f"sacc{uo}_{bt}")
                    for bt in range(BT)
                ]
                for t in range(DT):
                    for bt in range(BT):
                        nc.tensor.matmul(
                            saccs[bt],
                            xsT[:, t, bt * 128 : (bt + 1) * 128],
                            ksT[:, t, uo * NFREE : (uo + 1) * NFREE],
                            start=(t == 0),
                            stop=(t == DT - 1),
                        )
                for bt in range(BT):
                    nc.vector.tensor_add(
                        ysum[:, bt, uo * NFREE : (uo + 1) * NFREE],
                        saccs[bt],
                        bias_rep[:, uo * NFREE : (uo + 1) * NFREE],
                    )

            # ---- delta path: 7 rotated experts, ALL-fp8 DoubleRow
            def epilogue(acc, bt, n):
                # y[b,u] = sum_j acc[b,(u,j)]*dc[b,j] + ysum[b,u]
                tmp = epp.tile([128, DFREE], f32, tag="tmp")
                nc.vector.tensor_mul(tmp, acc, dc_rep[:, bt, :])
                yt = ypp.tile([128, UPT], f32, tag="yt")
                nc.vector.tensor_reduce(
                    yt,
                    tmp.rearrange("p (u k) -> p u k", k=KE),
                    axis=mybir.AxisListType.X,
                    op=mybir.AluOpType.add,
                )
                yf = ypp.tile([128, UPT], f32, tag="yf")
                nc.vector.tensor_add(yf, yt, ysum[:, bt, n * UPT : (n + 1) * UPT])
                nc.scalar.dma_start(
                    y[bt * 128 : (bt + 1) * 128, n * UPT : (n + 1) * UPT],
                    yf,
                )

            for g in range(NG):
                kt = ktp.tile([128, DT, NW], f8, tag="kt")
                for t in range(DT):
                    eng = nc.sync if t % 2 == 0 else nc.scalar
                    eng.dma_start(kt[:, t, :], kern8[:, t, g * NW : (g + 1) * NW])
                for half in range(NW // DFREE):
                    hs = half * DFREE
                    for bt in range(BT):
                        # full-bank tile, matmul writes the 448-col subview
                        accb = mps.tile([128, NFREE], f32, tag="acc", name="acc")
                        acc = accb[:, 0:DFREE]
                        for j in range(DT // 2):
                            nc.tensor.matmul(
                                acc,
                                x8T[:, 2 * j : 2 * j + 2, bt * 128 : (bt + 1) * 128],
                                kt[:, 2 * j : 2 * j + 2, hs : hs + DFREE],
                                start=(j == 0),
                                stop=(j == DT // 2 - 1),
                                perf_mode=mybir.MatmulPerfMode.DoubleRow,
                                skip_group_check=True,
                            )
                        epilogue(acc, bt, 2 * g + half)
    nc.compile()
    return nc


def _in_maps(x, c_prob, kernel, bias):
    import ml_dtypes

    bf16 = ml_dtypes.bfloat16
    e4m3 = ml_dtypes.float8_e4m3
    x = np.ascontiguousarray(x, dtype=np.float32)
    c_prob = np.ascontiguousarray(c_prob, dtype=np.float32)
    kernel = np.ascontiguousarray(kernel, dtype=np.float32)
    bias = np.ascontiguousarray(bias, dtype=np.float32)

    # orthogonal rotation with top right-singular vector of c_prob first
    _, _, vt = np.linalg.svd(c_prob, full_matrices=False)
    v = vt[0].astype(np.float32)
    if v.sum() < 0:
        v = -v
    M = np.concatenate([v[:, None], np.eye(K, dtype=np.float32)], axis=1)
    Qf, _ = np.linalg.qr(M)
    Q = Qf[:, :K].astype(np.float32)
    if np.dot(Q[:, 0], v) < 0:
        Q = -Q

    cr = c_prob @ Q  # [B, K]; cr[:,0] = a
    a = cr[:, 0]
    dc = cr[:, 1:] * np.float32(1.0 / (SX * SK))  # [B, KE]

    kr = (kernel.reshape(D * U, K) @ Q).reshape(D, U, K)
    kv_full = np.ascontiguousarray(kr[:, :, 0]).astype(bf16)  # [D, U]
    k7_full = (np.ascontiguousarray(kr[:, :, 1:]) * SK).astype(e4m3)  # [D, U, KE]
    xs_full = (x * a[:, None]).astype(bf16)
    x8_full = (x * SX).astype(e4m3)

    maps = []
    for c in range(NCORES):
        bq, uh = c % SHARD_B, c // SHARD_B
        bsl = slice(bq * BS, (bq + 1) * BS)
        usl = slice(uh * USH, (uh + 1) * USH)
        dcpk = np.ascontiguousarray(
            dc[bsl].reshape(BT, 128, KE).transpose(1, 0, 2).reshape(128, BT * KE)
        )
        x8p = np.ascontiguousarray(
            x8_full[bsl].T.reshape(DT, 128, BS).transpose(1, 0, 2)
        )
        ksp = np.ascontiguousarray(
            kv_full[:, usl].reshape(DT, 128, USH).transpose(1, 0, 2)
        )
        k8p = np.ascontiguousarray(
            k7_full[:, usl, :].reshape(DT, 128, UKS).transpose(1, 0, 2)
        )
        maps.append(
            {
                "xsum": np.ascontiguousarray(xs_full[bsl].T),
                "x8": x8p,
                "dcp": dcpk,
                "ksum": ksp,
                "kern8": k8p,
                "biasr": np.ascontiguousarray(
                    np.broadcast_to(bias[usl], (128, USH))
                ),
            }
        )
    return maps


def kernel(x, c_prob, kernel, bias):
    if "nc" not in _CACHE:
        _CACHE["nc"] = _build()
    nc = _CACHE["nc"]
    res = bass_utils.run_bass_kernel_spmd(
        nc, _in_maps(x, c_prob, kernel, bias), list(range(NCORES))
    )
    out = np.empty((B, U), dtype=np.float32)
    for c in range(NCORES):
        bq, uh = c % SHARD_B, c // SHARD_B
        out[bq * BS : (bq + 1) * BS, uh * USH : (uh + 1) * USH] = res.results[c]["y"]
    return out
